# revision 3
# baseline (speedup 1.0000x reference)
"""Trainium2 Bass kernel: dense transformer block (bilinear attention, no softmax).

Reference computation (B=2, S=2048, C=1024, H=16 heads, hd=64, HIDDEN=1024):
    q = split_heads(x @ Wq.T + bq) * hd**-0.5
    k = split_heads(x @ Wk.T + bk)
    v = split_heads(x @ Wv.T + bv)
    out = (q @ k.T) @ v          per (batch, head)   <-- no softmax!
    h = gelu(out @ W1.T + b1);  mlp = h @ W2.T + b2
    y = x + out + mlp

Key algebraic optimization: (q @ k.T) @ v == q @ (k.T @ v). k.T@v is a tiny
[64,64] per head, so attention drops from ~34 GFLOP to ~1 GFLOP.

Sharding (8 cores): rows (batch*seq = 4096) split 512/core; cores 0-3 hold
batch 0, cores 4-7 batch 1. Each core computes q/k/v/MLP for its rows only.
The only cross-core dependency is ktv = k.T@v (contraction over the full 2048
rows of a batch): each core computes its partial ktv and two compact 64KB
AllReduces over each 4-core batch group complete it.

Perf structure (vs the first working version):
  * A zero-byte-ish dummy AllReduce is issued as the first instruction: the
    runtime's one-time global barrier (which absorbs the 10-50us PJRT core
    start stagger) binds to it and overlaps the k/v compute phase instead of
    serializing before the first real AllReduce.
  * ktv AllReduce payload is compact [128, 4*64] per half (no block-diagonal
    zero padding); the reduced result is expanded on-chip into the zeroed
    block-diagonal tile with two strided DMAs per half.
  * The MLP (out@W1.T -> gelu -> @W2.T) runs in fp8 e4m3 DoubleRow perf mode
    (2 contraction chunks per instruction, 2x PE throughput). Weights are
    host-prescaled by 2^9 out of the fp8 subnormal range; the 2^-9 dequant is
    folded into the PSUM-eviction activations. Everything else runs fp16
    (same PE speed as bf16, 4x less rounding error) with fp32 PSUM.
  * Input DMAs are spread across the sync/scalar/vector/gpsimd queues with
    per-tile granularity on the critical path (first matmul needs only
    x-chunk0 + Wk-chunk0) and need-ordered issue.
"""

import sys
import types

sys.path.insert(0, "/opt/trn_rl_repo")

import numpy as np
import ml_dtypes

# ---------------------------------------------------------------------------
# NTFF profile hook shim (this image's antenv lacks axon_hooks; inject it so
# run_bass_kernel_spmd(trace=True) can profile). Harmless when unused.
# ---------------------------------------------------------------------------
if "antenv.axon_hooks" not in sys.modules:
    _m = types.ModuleType("antenv.axon_hooks")
    _m._hook = None
    _m.set_axon_ntff_profile_hook = lambda h: setattr(_m, "_hook", h)
    _m.get_axon_ntff_profile_hook = lambda: _m._hook
    sys.modules["antenv.axon_hooks"] = _m
    try:
        import antenv

        antenv.axon_hooks = _m
        from trn_agent_boot.trn_boot import _ntff_profile_via_ctypes

        _m.set_axon_ntff_profile_hook(
            _ntff_profile_via_ctypes("/opt/axon/libaxon_pjrt.so")
        )
    except Exception:
        pass

import concourse.bass as bass
import concourse.mybir as mybir
import concourse.tile as tile
from concourse import bacc
from concourse import bass_utils

bass_utils.upload_artifacts = lambda tmpdir: tmpdir  # no fish bucket here
from concourse.bass_utils import run_bass_kernel_spmd

F16 = mybir.dt.float16
F8 = mybir.dt.float8e4
F32 = mybir.dt.float32
AF = mybir.ActivationFunctionType
ALU = mybir.AluOpType
DR = mybir.MatmulPerfMode.DoubleRow

B, S, C = 2, 2048, 1024
NH, HD = 16, 64
SCALE = HD ** -0.5
NCORES = 8
R = (B * S) // NCORES        # 512 rows per core
P = 128
CH = C // P                  # 8 contraction chunks
RCH = R // P                 # 4 row chunks per core
HP = NH // 2                 # 8 head-pairs (one 128-partition chunk each)
HPH = HP // 2                # 4 head-pairs per AllReduce half
W8S = 512.0                  # fp8 weight prescale (2^9), exact in binary fp

_CACHE = {}


def _build(kv_bias: bool):
    """Build + compile the 8-core SPMD program. Returns the Bacc graph."""
    nc = bacc.Bacc("TRN2", target_bir_lowering=False, debug=False, num_devices=NCORES)

    # ---- DRAM I/O (per-core shapes; data differs per core) ----
    xtb_d = nc.dram_tensor("xtb", [P, CH * R], F16, kind="ExternalInput")
    wk0_d = nc.dram_tensor("wk0", [P, CH * 512], F16, kind="ExternalInput")
    wk1_d = nc.dram_tensor("wk1", [P, CH * 512], F16, kind="ExternalInput")
    wv0_d = nc.dram_tensor("wv0", [P, CH * 512], F16, kind="ExternalInput")
    wv1_d = nc.dram_tensor("wv1", [P, CH * 512], F16, kind="ExternalInput")
    wq_d = nc.dram_tensor("wq", [P, CH * C], F16, kind="ExternalInput")
    w1_d = nc.dram_tensor("w1", [P, CH * C], F8, kind="ExternalInput")
    w2_d = nc.dram_tensor("w2", [P, CH * C], F8, kind="ExternalInput")
    bqs_d = nc.dram_tensor("bqs", [P, CH], F32, kind="ExternalInput")
    b1r_d = nc.dram_tensor("b1r", [P, CH], F32, kind="ExternalInput")
    b2r_d = nc.dram_tensor("b2r", [P, CH], F32, kind="ExternalInput")
    if kv_bias:
        bkr_d = nc.dram_tensor("bkr", [1, C], F16, kind="ExternalInput")
        bvr_d = nc.dram_tensor("bvr", [1, C], F16, kind="ExternalInput")
    yt_d = nc.dram_tensor("yt", [P, CH * R], F16, kind="ExternalOutput")

    # Internal DRAM: dummy collective (absorbs the runtime's one-time global
    # barrier / core-start skew) + the two compact ktv AllReduces (64KB each).
    dum_in = nc.dram_tensor("dum_in", [1, 8], F32)
    dum_out = nc.dram_tensor("dum_out", [1, 8], F32)
    ktv_loc = [nc.dram_tensor(f"ktv_loc{i}", [P, HPH * HD], F16) for i in (0, 1)]
    ktv_red = [nc.dram_tensor(f"ktv_red{i}", [P, HPH * HD], F16) for i in (0, 1)]
    groups = [[0, 1, 2, 3], [4, 5, 6, 7]]

    with tile.TileContext(nc) as tc:
        with (
            tc.tile_pool(name="persist", bufs=1) as pp,
            tc.tile_pool(name="ypool", bufs=3) as yp,
            tc.tile_pool(name="psum", bufs=8, space="PSUM") as psp,
        ):
            # ---- persistent SBUF tiles ----
            xtb = [pp.tile([P, R], F16, name=f"xtb{c}") for c in range(CH)]
            wk0 = [pp.tile([P, 512], F16, name=f"wk0_{c}") for c in range(CH)]
            wv0 = [pp.tile([P, 512], F16, name=f"wv0_{c}") for c in range(CH)]
            wk1 = pp.tile([P, CH * 512], F16, name="wk1_sb")
            wv1 = pp.tile([P, CH * 512], F16, name="wv1_sb")
            wq = pp.tile([P, CH * C], F16, name="wq_sb")
            w1 = pp.tile([P, CH * C], F8, name="w1_sb")
            w2 = pp.tile([P, CH * C], F8, name="w2_sb")
            bqs = pp.tile([P, CH], F32, name="bqs_sb")
            b1r = pp.tile([P, CH], F32, name="b1r_sb")
            b2r = pp.tile([P, CH], F32, name="b2r_sb")
            k_sb = [pp.tile([P, C], F16, name=f"k_sb{i}") for i in range(RCH)]
            v_sb = [pp.tile([P, C], F16, name=f"v_sb{i}") for i in range(RCH)]
            q_sb = [pp.tile([P, R], F16, name=f"q_sb{i}") for i in range(HP)]
            out_b = [pp.tile([P, R], F16, name=f"out_b{i}") for i in range(HP)]
            out8 = [pp.tile([P, 2 * R], F8, name=f"out8_{i}") for i in range(HPH)]
            h8 = [pp.tile([P, 2 * R], F8, name=f"h8_{i}") for i in range(HPH)]
            xb2 = [pp.tile([P, R], F16, name=f"xb2_{i}") for i in range(CH)]
            ktv_acc = [
                pp.tile([P, HPH * HD], F16, name=f"ktv_acc{i}") for i in (0, 1)
            ]
            ktv_bb = pp.tile([P, HP * P], F16, name="ktv_bb")
            if kv_bias:
                ones = pp.tile([1, P], F16, name="ones_sb")
                bkr = pp.tile([1, C], F16, name="bkr_sb")
                bvr = pp.tile([1, C], F16, name="bvr_sb")

            w1_v = w1.rearrange("p (c f) -> p c f", c=CH)
            w2_v = w2.rearrange("p (c f) -> p c f", c=CH)
            out8_v = [t.rearrange("p (t r) -> p t r", t=2) for t in out8]
            h8_v = [t.rearrange("p (t r) -> p t r", t=2) for t in h8]
            bb_v = ktv_bb.rearrange("p (hp t d) -> p hp t d", hp=HP, t=2, d=HD)

            # ---- dummy collective: first instruction on the gpsimd queue.
            # The runtime's one-time pre-collective global barrier (which
            # waits for the slowest core to start) attaches here and runs
            # concurrently with the DMA prologue + k/v matmul phase.
            nc.gpsimd.collective_compute(
                "AllReduce",
                ALU.add,
                replica_groups=groups,
                ins=[dum_in[:]],
                outs=[dum_out[:]],
            )

            # ---- input DMAs, spread across queues in need-order ----
            # (only the sync/scalar/gpsimd queues can issue DMAs)
            for c in range(CH):
                nc.sync.dma_start(out=xtb[c][:], in_=xtb_d[:, c * R : (c + 1) * R])
            nc.sync.dma_start(out=wk1[:], in_=wk1_d[:])
            nc.sync.dma_start(out=wv1[:], in_=wv1_d[:])
            for c in range(CH):
                nc.scalar.dma_start(
                    out=wk0[c][:], in_=wk0_d[:, c * 512 : (c + 1) * 512]
                )
            nc.gpsimd.dma_start(out=bqs[:], in_=bqs_d[:])
            nc.gpsimd.dma_start(out=b1r[:], in_=b1r_d[:])
            nc.gpsimd.dma_start(out=b2r[:], in_=b2r_d[:])
            if kv_bias:
                nc.gpsimd.dma_start(out=bkr[:], in_=bkr_d[:])
                nc.gpsimd.dma_start(out=bvr[:], in_=bvr_d[:])
                nc.vector.memset(ones[:], 1.0)
            for c in range(CH):
                nc.gpsimd.dma_start(
                    out=wv0[c][:], in_=wv0_d[:, c * 512 : (c + 1) * 512]
                )
            nc.gpsimd.dma_start(out=wq[:], in_=wq_d[:])
            nc.gpsimd.dma_start(out=w1[:], in_=w1_d[:])
            nc.gpsimd.dma_start(out=w2[:], in_=w2_d[:])
            # zero the block-diagonal ktv tile (only diagonals get overwritten)
            nc.gpsimd.memset(ktv_bb[:], 0.0)
            # y-residual prep off the critical path: xb2[m] = x'[m] + b2
            for m in range(CH):
                nc.gpsimd.tensor_scalar(
                    xb2[m][:], xtb[m][:], b2r[:, m : m + 1], None, ALU.add
                )

            # ---- k, v projections (row-major [r, o]) ----
            # contraction-OUTER loops, split by output half (oh): compute
            # k(oh) then v(oh), then the compact partial ktv of that half,
            # and launch that half's AllReduce immediately.
            def proj_kv(w0_c, w1_t, brow, dst, oh):
                pss = [
                    psp.tile([P, 512], F32, name="ps", tag="ps")
                    for _ in range(RCH)
                ]
                for c in range(CH):
                    rhs = w0_c[c][:] if oh == 0 else w1_t[:, c * 512 : (c + 1) * 512]
                    for ri in range(RCH):
                        nc.tensor.matmul(
                            pss[ri][:],
                            xtb[c][:, ri * P : (ri + 1) * P],
                            rhs,
                            start=(c == 0),
                            stop=(c == CH - 1 and not kv_bias),
                        )
                for ri in range(RCH):
                    ps = pss[ri]
                    if kv_bias:
                        nc.tensor.matmul(
                            ps[:],
                            ones[:1, :],
                            brow[:1, oh * 512 : (oh + 1) * 512],
                            start=False,
                            stop=True,
                        )
                    dst_ap = dst[ri][:, oh * 512 : (oh + 1) * 512]
                    if ri % 2 == 0:
                        nc.vector.tensor_copy(dst_ap, ps[:])
                    else:
                        nc.scalar.activation(dst_ap, ps[:], AF.Copy)

            for oh in range(2):
                proj_kv(wk0, wk1, bkr if kv_bias else None, k_sb, oh)
                proj_kv(wv0, wv1, bvr if kv_bias else None, v_sb, oh)

                # partial ktv for this half: head-pairs packed [128,128] in
                # PSUM; the two 64-row diagonal strips are evicted into the
                # compact [128, 4*64] AllReduce payload (no zero padding).
                with tc.high_priority(offset=400):
                    pk = psp.tile([P, 512], F32, name="ps", tag="ps")
                    for hpl in range(HPH):
                        hp = oh * HPH + hpl
                        for ri in range(RCH):
                            nc.tensor.matmul(
                                pk[:, hpl * P : (hpl + 1) * P],
                                k_sb[ri][:, hp * P : (hp + 1) * P],
                                v_sb[ri][:, hp * P : (hp + 1) * P],
                                start=(ri == 0),
                                stop=(ri == RCH - 1),
                            )
                    pk_v = pk.rearrange("p (hp t d) -> p hp t d", hp=HPH, t=2, d=HD)
                    acc_v = ktv_acc[oh].rearrange("p (hp d) -> p hp d", hp=HPH, d=HD)
                    nc.vector.tensor_copy(acc_v[0:HD, :, :], pk_v[0:HD, :, 0, :])
                    nc.vector.tensor_copy(acc_v[HD:P, :, :], pk_v[HD:P, :, 1, :])
                    nc.scalar.dma_start(out=ktv_loc[oh][:], in_=ktv_acc[oh][:])
                    nc.gpsimd.collective_compute(
                        "AllReduce",
                        ALU.add,
                        replica_groups=groups,
                        ins=[ktv_loc[oh][:]],
                        outs=[ktv_red[oh][:]],
                    )

            # ---- q' projection (feature-major [o, r]), overlaps AllReduce ----
            for m in range(CH):
                ps = psp.tile([P, 512], F32, name="ps", tag="ps")
                for c in range(CH):
                    nc.tensor.matmul(
                        ps[:],
                        wq[:, c * C + m * P : c * C + (m + 1) * P],
                        xtb[c][:],
                        start=(c == 0),
                        stop=(c == CH - 1),
                    )
                nc.scalar.activation(
                    q_sb[m][:], ps[:], AF.Identity, bias=bqs[:, m : m + 1]
                )

            # ---- out' = blockdiag(ktv).T @ q', interleaved with MLP ----
            # Each reduced half is expanded into the zeroed block-diagonal
            # tile with two strided DMAs, then one [128,128]x[128,512] matmul
            # per head-pair. After the first half's out' chunks, the h'
            # contraction starts partially to overlap the second collective.
            def bb_load(oh):
                nc.sync.dma_start(
                    out=bb_v[0:HD, oh * HPH : (oh + 1) * HPH, 0, :],
                    in_=ktv_red[oh][0:HD, :],
                )
                nc.sync.dma_start(
                    out=bb_v[HD:P, oh * HPH : (oh + 1) * HPH, 1, :],
                    in_=ktv_red[oh][HD:P, :],
                )

            def out_chunk(hp):
                ps = psp.tile([P, 512], F32, name="ps", tag="ps")
                nc.tensor.matmul(
                    ps[:],
                    ktv_bb[:, hp * P : (hp + 1) * P],
                    q_sb[hp][:],
                    start=True,
                    stop=True,
                )
                nc.vector.tensor_copy(out_b[hp][:], ps[:])
                nc.vector.tensor_copy(out8_v[hp // 2][:, hp % 2, :], ps[:])

            def h_mm(ps, j, o2, start, stop):
                nc.tensor.matmul(
                    ps[:],
                    w1_v[:, 2 * o2 : 2 * o2 + 2, j * P : (j + 1) * P],
                    out8_v[o2][:, :, :],
                    start=start,
                    stop=stop,
                    perf_mode=DR,
                )

            def h_evict(ps, j):
                nc.scalar.activation(
                    h8_v[j // 2][:, j % 2, :],
                    ps[:],
                    AF.Gelu,
                    bias=b1r[:, j : j + 1],
                    scale=1.0 / W8S,
                )

            hps = []
            with tc.high_priority(offset=200):
                bb_load(0)
            for hp in range(HPH):
                out_chunk(hp)
            # h' partial: j-groups 0-5 over the available out pairs 0-1
            for j in range(6):
                ps = psp.tile([P, 512], F32, name="ps", tag="ps")
                hps.append(ps)
                for o2 in range(2):
                    h_mm(ps, j, o2, start=(o2 == 0), stop=False)
            with tc.high_priority(offset=200):
                bb_load(1)
            for hp in range(HPH, HP):
                out_chunk(hp)

            # ---- MLP hidden: h' = gelu(W1 out' + b1) (finish) ----
            for j in range(6):
                ps = hps[j]
                for o2 in range(2, CH // 2):
                    h_mm(ps, j, o2, start=False, stop=(o2 == CH // 2 - 1))
                h_evict(ps, j)
            for j in range(6, CH):
                ps = psp.tile([P, 512], F32, name="ps", tag="ps")
                for o2 in range(CH // 2):
                    h_mm(ps, j, o2, start=(o2 == 0), stop=(o2 == CH // 2 - 1))
                h_evict(ps, j)

            # ---- MLP out + residual: y' = (W2 h')/2^9 + out' + (x' + b2) ----
            for m in range(CH):
                ps = psp.tile([P, 512], F32, name="ps", tag="ps")
                for j2 in range(CH // 2):
                    nc.tensor.matmul(
                        ps[:],
                        w2_v[:, 2 * j2 : 2 * j2 + 2, m * P : (m + 1) * P],
                        h8_v[j2][:, :, :],
                        start=(j2 == 0),
                        stop=(j2 == CH // 2 - 1),
                        perf_mode=DR,
                    )
                y_t = yp.tile([P, R], F16, name="y_t")
                nc.vector.scalar_tensor_tensor(
                    y_t[:], ps[:], 1.0 / W8S, out_b[m][:], ALU.mult, ALU.add
                )
                nc.vector.tensor_add(y_t[:], y_t[:], xb2[m][:])
                nc.sync.dma_start(out=yt_d[:, m * R : (m + 1) * R], in_=y_t[:])

    nc.compile()
    return nc


def _get_nc(kv_bias: bool):
    key = ("nc", kv_bias)
    if key not in _CACHE:
        _CACHE[key] = _build(kv_bias)
    return _CACHE[key]


def _pack_pf(a):
    """[CH*P, F] row-major -> [P, CH*F] (partition-chunk packing)."""
    n, f = a.shape
    ch = n // P
    return np.ascontiguousarray(a.reshape(ch, P, f).transpose(1, 0, 2).reshape(P, ch * f))


def _split_halves(w_p):
    """[P, CH*C] chunk-major -> two [P, CH*512] (per-chunk column halves)."""
    v = w_p.reshape(P, CH, C)
    return (
        np.ascontiguousarray(v[:, :, 0:512].reshape(P, CH * 512)),
        np.ascontiguousarray(v[:, :, 512:C].reshape(P, CH * 512)),
    )


def _prep_inputs(x, Wq, bq, Wk, bk, Wv, bv, W1, b1, W2, b2, kv_bias):
    f16 = np.float16
    f8 = ml_dtypes.float8_e4m3
    wq_p = _pack_pf((Wq.T * SCALE).astype(np.float32)).astype(f16)
    wk_p = _pack_pf(np.ascontiguousarray(Wk.T)).astype(f16)
    wv_p = _pack_pf(np.ascontiguousarray(Wv.T)).astype(f16)
    wk0_p, wk1_p = _split_halves(wk_p)
    wv0_p, wv1_p = _split_halves(wv_p)
    w1_p = _pack_pf(np.ascontiguousarray(W1.T * W8S)).astype(f8)
    w2_p = _pack_pf(np.ascontiguousarray(W2.T * W8S)).astype(f8)
    bqs = np.ascontiguousarray((bq * SCALE).astype(np.float32).reshape(CH, P).T)
    b1r = np.ascontiguousarray(b1.astype(np.float32).reshape(CH, P).T)
    b2r = np.ascontiguousarray(b2.astype(np.float32).reshape(CH, P).T)

    xf = x.reshape(B * S, C)
    in_maps = []
    for core in range(NCORES):
        xs = xf[core * R : (core + 1) * R]           # [R, C]
        xt = _pack_pf(np.ascontiguousarray(xs.T))    # [P, CH*R] f32
        m = {
            "xtb": xt.astype(f16),
            "wk0": wk0_p,
            "wk1": wk1_p,
            "wv0": wv0_p,
            "wv1": wv1_p,
            "wq": wq_p,
            "w1": w1_p,
            "w2": w2_p,
            "bqs": bqs,
            "b1r": b1r,
            "b2r": b2r,
        }
        if kv_bias:
            m["bkr"] = bk.astype(f16).reshape(1, C)
            m["bvr"] = bv.astype(f16).reshape(1, C)
        in_maps.append(m)
    return in_maps


def _unpack_out(results):
    y = np.empty((B * S, C), np.float32)
    for core in range(NCORES):
        yt = np.asarray(results[core]["yt"]).astype(np.float32)  # [P, CH*R]
        blk = yt.reshape(P, CH, R).transpose(1, 0, 2).reshape(C, R)
        y[core * R : (core + 1) * R] = blk.T
    return y.reshape(B, S, C)


def _run(inputs, trace=False, trace_cores=None):
    x = np.asarray(inputs["x"], np.float32)
    args = [np.asarray(inputs[k], np.float32) for k in
            ("Wq", "bq", "Wk", "bk", "Wv", "bv", "W1", "b1", "W2", "b2")]
    kv_bias = bool(np.any(args[3]) or np.any(args[5]))
    nc = _get_nc(kv_bias)
    in_maps = _prep_inputs(x, *args, kv_bias)
    res = run_bass_kernel_spmd(
        nc, in_maps, core_ids=list(range(NCORES)), trace=trace,
        trace_cores=trace_cores,
    )
    return _unpack_out(res.results), res


def kernel(**inputs) -> np.ndarray:
    out, _ = _run(inputs, trace=False)
    return out


def kernel_profiled(**inputs):
    """Returns (output, exec_time_ns) using neuron-profile NTFF timing."""
    out, res = _run(inputs, trace=True)
    return out, res.exec_time_ns
